# revision 1
# baseline (speedup 1.0000x reference)
"""Trainium2 Bass kernel for DensityCalculator.

density[g] = sum_a sum_k aw[a,k]*exp(bw[a,k]*|g-x_a|^2) over a 64^3 grid,
then 3D FFT -> hamming filter -> inverse FFT -> normalize.

Distribution: grid x-axis sharded over 8 cores (8 x-planes each) for the
density phase; FFT is a distributed pencil decomposition (z,y transforms
local to the x-slab, AllToAll to kz-slabs, x transform + filter + inverse x,
AllToAll back, inverse y,z local).

Device algebra (separable Gaussian splatting):
 - the Gaussian factors per axis: aw*exp(bw*|g-x|^2) =
   prod_axis exp(bw*d_axis^2 + ln(aw)/3), so only 136 distinct 1-D
   coordinate offsets per core (8 slab-x + 64 y + 64 z) need exps:
   6 scalar-engine Exp ops over [128 atoms, 136] with per-partition
   scale=bw[:,k], bias=ln(aw[:,k])/3
 - G_k[a, (x,y)] = Ex_k (x) Ey_k outer product via one DVE broadcast
   multiply per k (bf16)
 - rho[z, (x,y)] = sum_k sum_a Ez_k[a,z] * G_k[a,(x,y)]: 6 accumulating
   bf16 matmuls with Ez_k as lhsT (atom contraction on the PE)
 - FFT as matmuls with 64x64 DFT matrices (fftshift/ifftshift folded into
   column/row permutations host-side), transposes on TensorE.
"""

import os
import sys
import numpy as np

for _p in ("/opt/trn_rl_repo", "/root/.axon_site", "/root/.axon_site/_ro/trn_rl_repo",
           "/root/.axon_site/_ro/pypackages"):
    if _p not in sys.path and os.path.isdir(_p):
        sys.path.append(_p)

import concourse.bass as bass
import concourse.tile as tile
from concourse import bacc, mybir
from concourse.bass_utils import run_bass_kernel_spmd

FP = mybir.dt.float32
FR = mybir.dt.float32r
BF = mybir.dt.bfloat16
Exp = mybir.ActivationFunctionType.Exp

N_CORES = 8
N = 64              # grid size per axis
A = 128             # atoms
K = 6               # gaussian terms
XL = 8              # x-planes per core
GPC = N * XL * N    # grid points per core (32768)
D1 = XL + 2 * N     # 1-D separable offsets per core (136: x-slab, y, z)

LAST_EXEC_NS = None
LAST_RESULTS = None
_COMPILED = None
DEBUG_TAPS = bool(os.environ.get("KERNEL_DEBUG_TAPS"))


def _build():
    nc = bacc.Bacc("TRN2", target_bir_lowering=False, debug=False,
                   num_devices=N_CORES)

    d2all = nc.dram_tensor("d2all", [A, D1], FP, kind="ExternalInput").ap()
    bwln = nc.dram_tensor("bwln", [A, 2 * K], FP, kind="ExternalInput").ap()
    mats = nc.dram_tensor("mats", [N, 128 + 7 * N], FR, kind="ExternalInput").ap()
    ham = nc.dram_tensor("ham", [N, 512], FR, kind="ExternalInput").ap()
    out = nc.dram_tensor("out", [N, 512], FP, kind="ExternalOutput").ap()
    taps = {}
    if DEBUG_TAPS:
        for nm in ("dbg_rho", "dbg_f1re", "dbg_f1im", "dbg_t1re", "dbg_t1im",
                   "dbg_f2re", "dbg_f2im", "dbg_g2re", "dbg_g2im",
                   "dbg_t2re", "dbg_t2im", "dbg_f3re", "dbg_f3im",
                   "dbg_g5re", "dbg_g5im"):
            taps[nm] = nc.dram_tensor(nm, [N, 512], FR, kind="ExternalOutput").ap()

    def tap(nm, tile_):
        if DEBUG_TAPS:
            nc.sync.dma_start(taps[nm], tile_[:])

    with tile.TileContext(nc) as tc:
        with tc.tile_pool(name="const", bufs=1) as constp, \
             tc.tile_pool(name="dram", bufs=1, space="DRAM") as dram:
            d2_sb = constp.tile([A, D1], FP)
            nc.sync.dma_start(d2_sb[:], d2all[:])
            bwln_sb = constp.tile([A, 2 * K], FP)
            nc.sync.dma_start(bwln_sb[:], bwln[:])
            mats_sb = constp.tile([N, 128 + 7 * N], FR)
            nc.scalar.dma_start(mats_sb[:], mats[:])
            ham_sb = constp.tile([N, 512], FR)
            nc.scalar.dma_start(ham_sb[:], ham[:])

            # (no warmup collective: the first real A2A pays the one-time
            # ncfw mesh setup directly; a warmup would only serialize in
            # front of it on the CC engine and delay it further.)

            # stationary views into mats
            Az2T = mats_sb[:, 0:128]
            c0 = 128
            ArT = mats_sb[:, c0:c0 + N]
            AiT = mats_sb[:, c0 + N:c0 + 2 * N]
            AiTn = mats_sb[:, c0 + 2 * N:c0 + 3 * N]
            BrT = mats_sb[:, c0 + 3 * N:c0 + 4 * N]
            BiT = mats_sb[:, c0 + 4 * N:c0 + 5 * N]
            BiTn = mats_sb[:, c0 + 5 * N:c0 + 6 * N]
            ident = mats_sb[:, c0 + 6 * N:c0 + 7 * N]
            # bf16 identity at both partition bases (0 and 64) so transposes
            # of the upper half of a stacked [128, .] tile pass the
            # base-partition match against their moving identity operand
            ident_bf2 = constp.tile([128, N], BF)
            nc.vector.tensor_copy(ident_bf2[0:N, :], ident)
            nc.vector.tensor_copy(ident_bf2[N:128, :], ident)

            # ---------------- Phase 1: separable density ----------------
            acc_pool = tc.tile_pool(name="p1acc", bufs=1, space="PSUM")
            accps = acc_pool.__enter__().tile([128, 512], FP, tag="acc")
            with tc.tile_pool(name="p1sb", bufs=1) as p1sb, \
                 tc.tile_pool(name="p1rho", bufs=1, space="PSUM") as p1rho:
                # E[a, k, :] = exp(bw_k*d2 + ln(aw_k)/3) over the 136 1-D
                # offsets; each of the three axis factors carries aw^(1/3)
                # so their product carries aw exactly once.
                E = p1sb.tile([A, K, D1], BF, tag="E")
                for k in range(K):
                    nc.scalar.activation(E[:, k, :], d2_sb[:], Exp,
                                         bias=bwln_sb[:, K + k:K + k + 1],
                                         scale=bwln_sb[:, k:k + 1])
                # G_k[a, x, y] = Ex_k (x) Ey_k  (DVE stride-0 broadcast mult)
                G = p1sb.tile([A, K, XL, N], BF, tag="G")
                for k in range(K):
                    nc.vector.tensor_tensor(
                        G[:, k],
                        E[:, k, 0:XL][:, :, None].broadcast_to([A, XL, N]),
                        E[:, k, XL:XL + N][:, None, :].broadcast_to([A, XL, N]),
                        op=mybir.AluOpType.mult)
                # rho[z, (x,y)] = sum_k Ez_k^T @ G_k  (atom contraction)
                rho_ps = p1rho.tile([N, 512], FP, tag="rho")
                for k in range(K):
                    nc.tensor.matmul(rho_ps[:],
                                     lhsT=E[:, k, XL + N:XL + 2 * N],
                                     rhs=G[:, k].rearrange("p x y -> p (x y)"),
                                     start=(k == 0), stop=(k == K - 1))
                rho_sb = p1sb.tile([N, 512], FR, tag="rho_sb")
                nc.vector.tensor_copy(rho_sb[:], rho_ps[:])
                tap("dbg_rho", rho_sb)
                # forward z-FFT: accps[(kz_re|kz_im), (x,y)] = Az2T^T @ rho
                nc.tensor.matmul(accps[:], lhsT=Az2T, rhs=rho_sb[:],
                                 start=True, stop=True)

            # ---------------- Phase 2: distributed FFT ----------------
            Copy = mybir.ActivationFunctionType.Copy

            def cpass(fsb, fpsp, sre, sim_n, sim_, re_in, im_in, stacked=False):
                """complex pass: re_out = sre@re + sim_n@im ; im_out = sim_@re + sre@im
                (sim_n = negated imag matrix). Round-copies out of PSUM, re on
                DVE and im on ScalarE so the two copies run concurrently.
                stacked=True returns one (128,512) bf16 tile [re | im] for A2A
                staging (bf16 wire format halves the collective bytes)."""
                ps_re = fpsp.tile([N, 512], FP, tag="psre")
                ps_im = fpsp.tile([N, 512], FP, tag="psim")
                nc.tensor.matmul(ps_re[:], lhsT=sre, rhs=re_in[:], start=True, stop=False)
                nc.tensor.matmul(ps_re[:], lhsT=sim_n, rhs=im_in[:], start=False, stop=True)
                nc.tensor.matmul(ps_im[:], lhsT=sim_, rhs=re_in[:], start=True, stop=False)
                nc.tensor.matmul(ps_im[:], lhsT=sre, rhs=im_in[:], start=False, stop=True)
                if stacked:
                    o = fsb.tile([128, 512], BF, tag="fstk")
                    nc.vector.tensor_copy(o[0:N, :], ps_re[:])
                    nc.scalar.activation(o[N:128, :], ps_im[:], Copy)
                    return o
                o_re = fsb.tile([N, 512], FR, tag="fre")
                o_im = fsb.tile([N, 512], FR, tag="fim")
                nc.vector.tensor_copy(o_re[:], ps_re[:])
                nc.scalar.activation(o_im[:], ps_im[:], Copy)
                return o_re, o_im

            dma_engines = (nc.sync, nc.scalar, nc.gpsimd)

            def tstage(fsb, tps, re_in, im_in, strided_out, src_block=None,
                       src_dt=FR):
                """transpose stage. Input [p | blk*64+q] (blk outer, q inner 64).
                strided_out=False: out[q | blk*64+p]  (contiguous 64-col writes)
                strided_out=True:  out[q | p*8+blk]   (stride-8 writes, 8 blocks)
                All 4 transposed blocks land in one PSUM tile so the
                PSUM->SBUF moves are 2 big strided copies instead of 8.
                src_block(src, t) may supply a custom (possibly permuted-view)
                [64, 128] AP for transpose block t; src_dt is its dtype.
                """
                o_re = fsb.tile([N, 512], FR, tag="tre")
                o_im = fsb.tile([N, 512], FR, tag="tim")
                for (src, dst, eng) in ((re_in, o_re, 0), (im_in, o_im, 1)):
                    pt = tps.tile([128, 4, N], src_dt, tag="pt")
                    for t in range(4):
                        blkap = (src[:, t * 128:(t + 1) * 128] if src_block is None
                                 else src_block(src, t))
                        if src_dt is FR:
                            idm = ident
                        else:
                            b = blkap.base_partition()
                            idm = ident_bf2[b:b + N, :]
                        nc.tensor.transpose(pt[:, t, :], blkap, idm)
                    # block blk = 2t+h: dst cols from pt[64h:64h+64, t, :]
                    # re-copies on DVE, im-copies on ScalarE (concurrent)
                    if strided_out:
                        dstv = dst.rearrange("w (i t h) -> w t i h", i=N, t=4, h=2)
                        for h in range(2):
                            if eng == 0:
                                nc.vector.tensor_copy(dstv[:, :, :, h],
                                                      pt[h * N:(h + 1) * N])
                            else:
                                nc.scalar.activation(dstv[:, :, :, h],
                                                     pt[h * N:(h + 1) * N], Copy)
                    else:
                        dstv = dst.rearrange("w (t h i) -> w t h i", t=4, h=2, i=N)
                        for h in range(2):
                            if eng == 0:
                                nc.vector.tensor_copy(dstv[:, :, h],
                                                      pt[h * N:(h + 1) * N])
                            else:
                                nc.scalar.activation(dstv[:, :, h],
                                                     pt[h * N:(h + 1) * N], Copy)
                return o_re, o_im

            fsb_pool = tc.tile_pool(name="fft", bufs=2)
            fsb = fsb_pool.__enter__()
            # forward z was computed in phase 1; round-copy out
            f_re = fsb.tile([N, 512], FR, tag="fre")
            f_im = fsb.tile([N, 512], FR, tag="fim")
            nc.vector.tensor_copy(f_re[:], accps[0:N, :])
            nc.vector.tensor_copy(f_im[:], accps[N:128, :])
            acc_pool.__exit__(None, None, None)
            with tc.tile_pool(name="fps", bufs=2, space="PSUM") as fps, \
                 tc.tile_pool(name="ps6p", bufs=1, space="PSUM") as ps6p, \
                 tc.tile_pool(name="tps", bufs=3, space="PSUM") as tps:
                tap("dbg_f1re", f_re); tap("dbg_f1im", f_im)
                t_re, t_im = tstage(fsb, tps, f_re, f_im, True)   # [y | kz*8+xl]
                tap("dbg_t1re", t_re); tap("dbg_t1im", t_im)
                f2s = cpass(fsb, fps, ArT, AiTn, AiT, t_re, t_im,
                            stacked=True)                 # [ky± | kz*8+xl]

                # A2A #1: -> [ky | kzl*64 + x]   (chunk for dest d = cols [64d,64d+64))
                a_in = dram.tile([N_CORES, 2, N, 8, 8], BF, tag="a2a_in")
                a_out = dram.tile([N_CORES, 2, N, 8, 8], BF, tag="a2a_out")
                for dd in range(N_CORES):
                    dma_engines[dd % 3].dma_start(
                        a_in[dd].rearrange("q p kl xl -> (q p) kl xl"),
                        f2s[:, dd * N:(dd + 1) * N].rearrange(
                            "p (kl xl) -> p kl xl", kl=8, xl=8))
                nc.gpsimd.collective_compute(
                    "AllToAll", mybir.AluOpType.bypass,
                    replica_groups=[list(range(N_CORES))],
                    ins=[a_in.opt()], outs=[a_out.opt()])
                # recv contiguously (cheap descriptors), then DVE-permute
                # [p | s*64+kl*8+xl] -> [p | kl*64+s*8+xl]
                g_raw = fsb.tile([128, 512], BF, tag="graw")
                for ss in range(N_CORES):
                    dma_engines[ss % 3].dma_start(
                        g_raw[:, ss * N:(ss + 1) * N].rearrange(
                            "p (kl xl) -> p kl xl", kl=8, xl=8),
                        a_out[ss].rearrange("q p kl xl -> (q p) kl xl"))

                g_re = fsb.tile([N, 512], FR, tag="fre")
                g_im = fsb.tile([N, 512], FR, tag="fim")
                nc.vector.tensor_copy(
                    g_re.rearrange("p (kl s xl) -> p s kl xl", kl=8, s=8, xl=8),
                    g_raw[0:N, :].rearrange("p (s kl xl) -> p s kl xl", s=8, kl=8, xl=8))
                nc.scalar.activation(
                    g_im.rearrange("p (kl s xl) -> p s kl xl", kl=8, s=8, xl=8),
                    g_raw[N:128, :].rearrange("p (s kl xl) -> p s kl xl", s=8, kl=8, xl=8),
                    Copy)
                t2_re, t2_im = tstage(fsb, tps, g_re, g_im, False)  # [x | kzl*64+ky]
                tap("dbg_t2re", t2_re); tap("dbg_t2im", t2_im)
                # P3 forward x, then filter fused into the PSUM->SBUF copy
                ps3_re = fps.tile([N, 512], FP, tag="psre")
                ps3_im = fps.tile([N, 512], FP, tag="psim")
                nc.tensor.matmul(ps3_re[:], lhsT=ArT, rhs=t2_re[:], start=True, stop=False)
                nc.tensor.matmul(ps3_re[:], lhsT=AiTn, rhs=t2_im[:], start=False, stop=True)
                nc.tensor.matmul(ps3_im[:], lhsT=AiT, rhs=t2_re[:], start=True, stop=False)
                nc.tensor.matmul(ps3_im[:], lhsT=ArT, rhs=t2_im[:], start=False, stop=True)
                f3_re = fsb.tile([N, 512], FR, tag="fre")
                f3_im = fsb.tile([N, 512], FR, tag="fim")
                nc.vector.tensor_tensor(f3_re[:], ps3_re[:], ham_sb[:], op=mybir.AluOpType.mult)
                nc.vector.tensor_tensor(f3_im[:], ps3_im[:], ham_sb[:], op=mybir.AluOpType.mult)

                tap("dbg_f3re", f3_re); tap("dbg_f3im", f3_im)
                g4_re, g4_im = cpass(fsb, fps, BrT, BiTn, BiT, f3_re, f3_im)  # [x | kzl*64+ky]
                t3_re, t3_im = tstage(fsb, tps, g4_re, g4_im, True)  # [ky | x*8+kzl]
                f5s = cpass(fsb, fps, BrT, BiTn, BiT, t3_re, t3_im, stacked=True)  # [y± | x*8+kzl]

                # A2A #2: -> [y | xl*64 + kz]   (chunk for dest d = cols [64d,64d+64))
                a2_in = dram.tile([N_CORES, 2, N, 8, 8], BF, tag="a2a2_in")
                a2_out = dram.tile([N_CORES, 2, N, 8, 8], BF, tag="a2a2_out")
                for dd in range(N_CORES):
                    dma_engines[dd % 3].dma_start(
                        a2_in[dd].rearrange("q p xl kl -> (q p) xl kl"),
                        f5s[:, dd * N:(dd + 1) * N].rearrange("p (xl kl) -> p xl kl", xl=8, kl=8))
                nc.gpsimd.collective_compute(
                    "AllToAll", mybir.AluOpType.bypass,
                    replica_groups=[list(range(N_CORES))],
                    ins=[a2_in.opt()], outs=[a2_out.opt()])
                # recv contiguously, then DVE-permute [p | s*64+xl*8+kl] -> [p | xl*64+s*8+kl]
                g5_raw = fsb.tile([128, 512], BF, tag="graw")
                for ss in range(N_CORES):
                    dma_engines[ss % 3].dma_start(
                        g5_raw[:, ss * N:(ss + 1) * N].rearrange("p (xl kl) -> p xl kl", xl=8, kl=8),
                        a2_out[ss].rearrange("q p xl kl -> (q p) xl kl"))
                g5_re = fsb.tile([N, 512], FR, tag="fre")
                g5_im = fsb.tile([N, 512], FR, tag="fim")
                nc.vector.tensor_copy(
                    g5_re.rearrange("p (xl s kl) -> p s xl kl", xl=8, s=8, kl=8),
                    g5_raw[0:N, :].rearrange("p (s xl kl) -> p s xl kl", s=8, xl=8, kl=8))
                nc.scalar.activation(
                    g5_im.rearrange("p (xl s kl) -> p s xl kl", xl=8, s=8, kl=8),
                    g5_raw[N:128, :].rearrange("p (s xl kl) -> p s xl kl", s=8, xl=8, kl=8),
                    Copy)
                t4_re, t4_im = tstage(fsb, tps, g5_re, g5_im, False)  # [kz | xl*64+y]
                # P6: inverse z, real part only
                ps6 = ps6p.tile([N, 512], FP, tag="ps6")
                nc.tensor.matmul(ps6[:], lhsT=BrT, rhs=t4_re[:], start=True, stop=False)
                nc.tensor.matmul(ps6[:], lhsT=BiTn, rhs=t4_im[:], start=False, stop=True)
                out_sb = fsb.tile([N, 512], FP, tag="osb")
                nc.vector.tensor_copy(out_sb[:], ps6[:])
                nc.sync.dma_start(out[:], out_sb[:])
            fsb_pool.__exit__(None, None, None)

    nc.compile()
    return nc


def _get_compiled():
    global _COMPILED
    if _COMPILED is None:
        _COMPILED = _build()
    return _COMPILED


def _host_inputs(X, aw, bw, real_grid_flat, hamming):
    X = np.asarray(X, np.float32)
    aw = np.asarray(aw, np.float32)
    bw = np.asarray(bw, np.float32)
    grid = np.asarray(real_grid_flat, np.float32)
    hamming = np.asarray(hamming, np.float32)

    arr = grid.reshape(N, N, N, 3)                       # [x, y, z, 3]
    xs = arr[:, 0, 0, 0]                                 # (64,)
    ys = arr[0, :, 0, 1]
    zs = arr[0, 0, :, 2]
    d2y = (ys[None, :] - X[:, 1:2]) ** 2                 # (128, 64)
    d2z = (zs[None, :] - X[:, 2:3]) ** 2                 # (128, 64)

    lnaw3 = (np.log(np.maximum(aw, 1e-38)) / 3.0).astype(np.float32)
    bwln = np.concatenate([bw, lnaw3], 1).astype(np.float32)    # (128, 12)

    F = np.fft.fft(np.eye(N), axis=0, norm='ortho')
    IF = np.fft.ifft(np.eye(N), axis=0, norm='ortho')
    perm = (np.arange(N) + N // 2) % N
    Am = F[:, perm]
    Bm = IF[perm, :]
    Ar, Ai = Am.real.astype(np.float32), Am.imag.astype(np.float32)
    Br, Bi = Bm.real.astype(np.float32), Bm.imag.astype(np.float32)
    Az2T = np.concatenate([Ar.T, Ai.T], 1)               # (64, 128)
    mats = np.concatenate(
        [Az2T, Ar.T, Ai.T, -Ai.T, Br.T, Bi.T, -Bi.T,
         np.eye(N, dtype=np.float32)], 1)                # (64, 576)

    Hfull = np.fft.ifftshift(hamming)                    # [kx, ky, kz]

    in_maps = []
    for c in range(N_CORES):
        d2x = (xs[None, 8 * c:8 * (c + 1)] - X[:, 0:1]) ** 2    # (128, 8)
        d2all = np.concatenate([d2x, d2y, d2z], 1).astype(np.float32)
        Hc = np.ascontiguousarray(
            np.transpose(Hfull[:, :, 8 * c:8 * (c + 1)], (0, 2, 1))).reshape(N, 512)
        in_maps.append({"d2all": d2all, "bwln": bwln,
                        "mats": mats, "ham": Hc})
    return in_maps


def kernel(X, aw, bw, real_grid_flat, hamming):
    global LAST_EXEC_NS, LAST_RESULTS
    in_maps = _host_inputs(X, aw, bw, real_grid_flat, hamming)
    nc = _get_compiled()

    trace = bool(os.environ.get("BASS_TRACE"))
    res = run_bass_kernel_spmd(nc, in_maps, core_ids=list(range(N_CORES)),
                               trace=trace)
    LAST_EXEC_NS = res.exec_time_ns
    global LAST_RESULTS
    LAST_RESULTS = res.results

    full = np.empty((N, N, N), np.float32)               # [z, x, y]
    for c in range(N_CORES):
        full[:, 8 * c:8 * (c + 1), :] = res.results[c]["out"].reshape(N, 8, N)
    o = np.transpose(full, (1, 2, 0))                    # [x, y, z]
    o = (o - o.mean()) / (o.std() + 1e-8)
    return o.astype(np.float32)



# revision 3
# speedup vs baseline: 1.0865x; 1.0865x over previous
"""Trainium2 Bass kernel for DensityCalculator.

density[g] = sum_a sum_k aw[a,k]*exp(bw[a,k]*|g-x_a|^2) over a 64^3 grid,
then 3D FFT -> hamming filter -> inverse FFT -> normalize.

Distribution: grid x-axis sharded over 8 cores (8 x-planes each) for the
density phase; FFT is a distributed pencil decomposition (z,y transforms
local to the x-slab, AllToAll to kz-slabs, x transform + filter + inverse x,
AllToAll back, inverse y,z local).

Device algebra:
 - separable Gaussian splatting: 6 scalar-engine Exp ops over
   [128 atoms, 136 1-D offsets], DVE outer products, 6 accumulating bf16
   matmuls contracting atoms on the PE.
 - FFT as matmuls with DFT matrices (fftshift/ifftshift folded into
   permutations host-side). Complex passes are ONE fused 128x128 block
   matmul per stage: rhs carries [re(64); im(64)] stacked on partitions,
   lhsT = [[Mr.T, Mi.T], [-Mi.T, Mr.T]].
 - transposes on TensorE over the stacked halves; all FFT-stage tiles are
   bf16 (fp32 PSUM accumulation keeps the products exact).
 - one DMA per A2A stage/recv and per input group (2-D access patterns)
   to minimize DMA queue count (queue drains dominate the NEFF epilogue).
"""

import os
import sys
import numpy as np

for _p in ("/opt/trn_rl_repo", "/root/.axon_site", "/root/.axon_site/_ro/trn_rl_repo",
           "/root/.axon_site/_ro/pypackages"):
    if _p not in sys.path and os.path.isdir(_p):
        sys.path.append(_p)

import concourse.bass as bass
import concourse.tile as tile
from concourse import bacc, mybir
from concourse.bass_utils import run_bass_kernel_spmd

FP = mybir.dt.float32
FR = mybir.dt.float32r
BF = mybir.dt.bfloat16
Exp = mybir.ActivationFunctionType.Exp
Copy = mybir.ActivationFunctionType.Copy

N_CORES = 8
N = 64              # grid size per axis
A = 128             # atoms
K = 6               # gaussian terms
XL = 8              # x-planes per core
D1 = XL + 2 * N     # 1-D separable offsets per core (136: x-slab, y, z)

LAST_EXEC_NS = None
LAST_RESULTS = None
LAST_BKR = None
_COMPILED = None


def _build():
    nc = bacc.Bacc("TRN2", target_bir_lowering=False, debug=False,
                   num_devices=N_CORES)

    # din: [d2all(136) | bwln(12)]
    din = nc.dram_tensor("din", [A, D1 + 2 * K], FP, kind="ExternalInput").ap()
    # cmats: [WA(128) | WB(128) | WBre(64) | H2(512) | ident(64) | Az2T(128)]
    cmats = nc.dram_tensor("cmats", [128, 1024], FR, kind="ExternalInput").ap()
    out = nc.dram_tensor("out", [N, 512], FP, kind="ExternalOutput").ap()

    with tile.TileContext(nc) as tc:
        with tc.tile_pool(name="const", bufs=1) as constp, \
             tc.tile_pool(name="dram", bufs=1, space="DRAM") as dram:
            din_sb = constp.tile([A, D1 + 2 * K], FP)
            nc.sync.dma_start(din_sb[:], din[:])
            cm_sb = constp.tile([128, 1024], FR)
            nc.scalar.dma_start(cm_sb[:], cmats[:])

            d2_sb = din_sb[:, 0:D1]
            bwln_sb = din_sb[:, D1:D1 + 2 * K]
            WAv = cm_sb[:, 0:128]
            WBv = cm_sb[:, 128:256]
            WBrev = cm_sb[:, 256:320]
            H2v = cm_sb[:, 320:832]
            identv = cm_sb[0:N, 832:896]
            Az2Tv = cm_sb[0:N, 896:1024]

            # bf16 copies of the stationary matrices
            WA_bf = constp.tile([128, 128], BF)
            WB_bf = constp.tile([128, 128], BF)
            WBre_bf = constp.tile([128, N], BF)
            Az_bf = constp.tile([N, 128], BF)
            ident_bf2 = constp.tile([128, N], BF)
            nc.vector.tensor_copy(WA_bf[:], WAv)
            nc.vector.tensor_copy(WB_bf[:], WBv)
            nc.scalar.activation(WBre_bf[:], WBrev, Copy)
            nc.scalar.activation(Az_bf[:], Az2Tv, Copy)
            nc.vector.tensor_copy(ident_bf2[0:N, :], identv)
            nc.vector.tensor_copy(ident_bf2[N:128, :], identv)

            # ---------------- Phase 1: separable density ----------------
            acc_pool = tc.tile_pool(name="p1acc", bufs=1, space="PSUM")
            accps = acc_pool.__enter__().tile([128, 512], FP, tag="acc")
            with tc.tile_pool(name="p1sb", bufs=1) as p1sb, \
                 tc.tile_pool(name="p1rho", bufs=1, space="PSUM") as p1rho:
                # E[a, k, :] = exp(bw_k*d2 + ln(aw_k)/3) over the 136 1-D
                # offsets; each axis factor carries aw^(1/3).
                E = p1sb.tile([A, K, D1], BF, tag="E")
                for k in range(K):
                    nc.scalar.activation(E[:, k, :], d2_sb, Exp,
                                         bias=bwln_sb[:, K + k:K + k + 1],
                                         scale=bwln_sb[:, k:k + 1])
                # G_k[a, x, y] = Ex_k (x) Ey_k  (DVE stride-0 broadcast mult)
                G = p1sb.tile([A, K, XL, N], BF, tag="G")
                for k in range(K):
                    nc.vector.tensor_tensor(
                        G[:, k],
                        E[:, k, 0:XL][:, :, None].broadcast_to([A, XL, N]),
                        E[:, k, XL:XL + N][:, None, :].broadcast_to([A, XL, N]),
                        op=mybir.AluOpType.mult)
                # rho[z, (x,y)] = sum_k Ez_k^T @ G_k  (atom contraction)
                rho_ps = p1rho.tile([N, 512], FP, tag="rho")
                for k in range(K):
                    nc.tensor.matmul(rho_ps[:],
                                     lhsT=E[:, k, XL + N:XL + 2 * N],
                                     rhs=G[:, k].rearrange("p x y -> p (x y)"),
                                     start=(k == 0), stop=(k == K - 1))
                rho_sb = p1sb.tile([N, 512], BF, tag="rho_sb")
                nc.vector.tensor_copy(rho_sb[:], rho_ps[:])
                # forward z-FFT: accps[(kz_re|kz_im), (x,y)] = Az2T^T @ rho
                nc.tensor.matmul(accps[:], lhsT=Az_bf[:], rhs=rho_sb[:],
                                 start=True, stop=True)

            # ---------------- Phase 2: distributed FFT ----------------
            fsb_pool = tc.tile_pool(name="fft", bufs=2)
            fsb = fsb_pool.__enter__()

            def cpass(W, s_in, tag="cstk"):
                """fused complex pass: one 128x128 block matmul over the
                stacked [re;im] rhs; PSUM round-copies split DVE/ScalarE."""
                ps = fps.tile([128, 512], FP, tag="ps")
                nc.tensor.matmul(ps[:], lhsT=W[:], rhs=s_in[:],
                                 start=True, stop=True)
                o = fsb.tile([128, 512], BF, tag=tag)
                nc.vector.tensor_copy(o[0:N, :], ps[0:N, :])
                nc.scalar.activation(o[N:128, :], ps[N:128, :], Copy)
                return o

            def tstage(s_in, strided_out):
                """transpose stage on a stacked [128,512] bf16 tile.
                Per half (re base 0 / im base 64), input [p | blk*64+q]:
                strided_out=False: out[q | blk*64+p]
                strided_out=True:  out[q | p*8+blk]
                All 8 transposed blocks land in one PSUM bank; the
                PSUM->SBUF moves are 4 strided copies (re on DVE, im on
                ScalarE, concurrently)."""
                o = fsb.tile([128, 512], BF, tag="tstk")
                pt = tps.tile([128, 2, 4, N], BF, tag="pt")
                for half in range(2):
                    src = s_in[half * N:(half + 1) * N, :]
                    for t in range(4):
                        blkap = src[:, t * 128:(t + 1) * 128]
                        b = blkap.base_partition()
                        nc.tensor.transpose(pt[:, half, t, :], blkap,
                                            ident_bf2[b:b + N, :])
                for half in range(2):
                    dst = o[half * N:(half + 1) * N, :]
                    ptv = pt[:, half]          # [128, 4, 64]
                    if strided_out:
                        dstv = dst.rearrange("w (i t h) -> w t i h",
                                             i=N, t=4, h=2)
                        for h in range(2):
                            if half == 0:
                                nc.vector.tensor_copy(
                                    dstv[:, :, :, h], ptv[h * N:(h + 1) * N])
                            else:
                                nc.scalar.activation(
                                    dstv[:, :, :, h], ptv[h * N:(h + 1) * N],
                                    Copy)
                    else:
                        dstv = dst.rearrange("w (t h i) -> w t h i",
                                             t=4, h=2, i=N)
                        for h in range(2):
                            if half == 0:
                                nc.vector.tensor_copy(
                                    dstv[:, :, h], ptv[h * N:(h + 1) * N])
                            else:
                                nc.scalar.activation(
                                    dstv[:, :, h], ptv[h * N:(h + 1) * N],
                                    Copy)
                return o

            def unpermute(raw, tag, kl_outer):
                """A2A recv reorder [p | s*64 + a*8 + b] -> [p | a*64 + s*8 + b]
                on the stacked halves (re on DVE, im on ScalarE)."""
                g = fsb.tile([128, 512], BF, tag=tag)
                for half in range(2):
                    lo, hi = half * N, (half + 1) * N
                    dstv = g[lo:hi, :].rearrange(
                        "p (a s b) -> p s a b", a=8, s=8, b=8)
                    srcv = raw[lo:hi, :].rearrange(
                        "p (s a b) -> p s a b", s=8, a=8, b=8)
                    if half == 0:
                        nc.vector.tensor_copy(dstv, srcv)
                    else:
                        nc.scalar.activation(dstv, srcv, Copy)
                return g

            # forward z was accumulated in phase 1; round-copy out stacked
            f1 = fsb.tile([128, 512], BF, tag="f1")
            nc.vector.tensor_copy(f1[0:N, :], accps[0:N, :])
            nc.scalar.activation(f1[N:128, :], accps[N:128, :], Copy)
            acc_pool.__exit__(None, None, None)

            with tc.tile_pool(name="fps", bufs=2, space="PSUM") as fps, \
                 tc.tile_pool(name="tps", bufs=3, space="PSUM") as tps:
                t1 = tstage(f1, True)              # [y± | kz*8+xl]
                f2s = cpass(WA_bf, t1)             # [ky± | kz*8+xl]

                # A2A #1: -> [ky± | kzl*64 + x]  (chunk d = cols [64d,64d+64))
                a_in = dram.tile([N_CORES, 2, N, 8, 8], BF, tag="a2a_in")
                a_out = dram.tile([N_CORES, 2, N, 8, 8], BF, tag="a2a_out")
                nc.sync.dma_start(
                    a_in.rearrange("d q p kl xl -> (q p) d (kl xl)"),
                    f2s.rearrange("p (d c) -> p d c", d=8))
                nc.gpsimd.collective_compute(
                    "AllToAll", mybir.AluOpType.bypass,
                    replica_groups=[list(range(N_CORES))],
                    ins=[a_in.opt()], outs=[a_out.opt()])
                g_raw = fsb.tile([128, 512], BF, tag="graw")
                nc.sync.dma_start(
                    g_raw.rearrange("p (s c) -> p s c", s=8),
                    a_out.rearrange("s q p kl xl -> (q p) s (kl xl)"))
                g = unpermute(g_raw, "g", True)    # [ky± | kl*64 + s*8+xl]

                t2 = tstage(g, False)              # [x± | kzl*64+ky]
                # P3 forward x, filter fused into the PSUM->SBUF copy
                ps3 = fps.tile([128, 512], FP, tag="ps")
                nc.tensor.matmul(ps3[:], lhsT=WA_bf[:], rhs=t2[:],
                                 start=True, stop=True)
                f3 = fsb.tile([128, 512], BF, tag="f3")
                nc.vector.tensor_tensor(f3[:], ps3[:], H2v,
                                        op=mybir.AluOpType.mult)
                g4 = cpass(WB_bf, f3)              # [x± | kzl*64+ky]
                t3 = tstage(g4, True)              # [ky± | x*8+kzl]
                f5s = cpass(WB_bf, t3)             # [y± | x*8+kzl]

                # A2A #2: -> [y± | xl*64 + kz]  (chunk d = cols [64d,64d+64))
                a2_in = dram.tile([N_CORES, 2, N, 8, 8], BF, tag="a2a2_in")
                a2_out = dram.tile([N_CORES, 2, N, 8, 8], BF, tag="a2a2_out")
                nc.sync.dma_start(
                    a2_in.rearrange("d q p xl kl -> (q p) d (xl kl)"),
                    f5s.rearrange("p (d c) -> p d c", d=8))
                nc.gpsimd.collective_compute(
                    "AllToAll", mybir.AluOpType.bypass,
                    replica_groups=[list(range(N_CORES))],
                    ins=[a2_in.opt()], outs=[a2_out.opt()])
                g5_raw = fsb.tile([128, 512], BF, tag="graw")
                nc.sync.dma_start(
                    g5_raw.rearrange("p (s c) -> p s c", s=8),
                    a2_out.rearrange("s q p xl kl -> (q p) s (xl kl)"))
                g5 = unpermute(g5_raw, "g", False)  # [y± | xl*64 + s*8+kl]

                t4 = tstage(g5, False)             # [kz± | xl*64+y]
                # P6: inverse z, real part only
                ps6 = fps.tile([N, 512], FP, tag="ps6")
                nc.tensor.matmul(ps6[:], lhsT=WBre_bf[:], rhs=t4[:],
                                 start=True, stop=True)
                out_sb = fsb.tile([N, 512], FP, tag="osb")
                nc.vector.tensor_copy(out_sb[:], ps6[:])
                nc.sync.dma_start(out[:], out_sb[:])
            fsb_pool.__exit__(None, None, None)

    nc.compile()
    return nc


def _get_compiled():
    global _COMPILED
    if _COMPILED is None:
        _COMPILED = _build()
    return _COMPILED


def _host_inputs(X, aw, bw, real_grid_flat, hamming):
    X = np.asarray(X, np.float32)
    aw = np.asarray(aw, np.float32)
    bw = np.asarray(bw, np.float32)
    grid = np.asarray(real_grid_flat, np.float32)
    hamming = np.asarray(hamming, np.float32)

    arr = grid.reshape(N, N, N, 3)                       # [x, y, z, 3]
    xs = arr[:, 0, 0, 0]                                 # (64,)
    ys = arr[0, :, 0, 1]
    zs = arr[0, 0, :, 2]
    d2y = (ys[None, :] - X[:, 1:2]) ** 2                 # (128, 64)
    d2z = (zs[None, :] - X[:, 2:3]) ** 2                 # (128, 64)

    lnaw3 = (np.log(np.maximum(aw, 1e-38)) / 3.0).astype(np.float32)
    bwln = np.concatenate([bw, lnaw3], 1).astype(np.float32)    # (128, 12)

    F = np.fft.fft(np.eye(N), axis=0, norm='ortho')
    IF = np.fft.ifft(np.eye(N), axis=0, norm='ortho')
    perm = (np.arange(N) + N // 2) % N
    Am = F[:, perm]
    Bm = IF[perm, :]
    Ar, Ai = Am.real.astype(np.float32), Am.imag.astype(np.float32)
    Br, Bi = Bm.real.astype(np.float32), Bm.imag.astype(np.float32)
    WA_T = np.block([[Ar.T, Ai.T], [-Ai.T, Ar.T]]).astype(np.float32)
    WB_T = np.block([[Br.T, Bi.T], [-Bi.T, Br.T]]).astype(np.float32)
    WBre_T = np.concatenate([Br.T, -Bi.T], 0).astype(np.float32)  # (128, 64)
    Az2T = np.concatenate([Ar.T, Ai.T], 1)               # (64, 128)
    Az2Tp = np.zeros((128, 128), np.float32)
    Az2Tp[0:N, :] = Az2T
    identp = np.zeros((128, N), np.float32)
    identp[0:N, :] = np.eye(N, dtype=np.float32)

    Hfull = np.fft.ifftshift(hamming)                    # [kx, ky, kz]

    in_maps = []
    for c in range(N_CORES):
        d2x = (xs[None, 8 * c:8 * (c + 1)] - X[:, 0:1]) ** 2    # (128, 8)
        d2all = np.concatenate([d2x, d2y, d2z], 1).astype(np.float32)
        din = np.concatenate([d2all, bwln], 1).astype(np.float32)
        Hc = np.ascontiguousarray(
            np.transpose(Hfull[:, :, 8 * c:8 * (c + 1)], (0, 2, 1))).reshape(N, 512)
        H2c = np.concatenate([Hc, Hc], 0).astype(np.float32)     # (128, 512)
        cm = np.concatenate(
            [WA_T, WB_T, WBre_T, H2c, identp, Az2Tp], 1).astype(np.float32)
        in_maps.append({"din": din, "cmats": cm})
    return in_maps


def kernel(X, aw, bw, real_grid_flat, hamming):
    global LAST_EXEC_NS, LAST_RESULTS, LAST_BKR
    in_maps = _host_inputs(X, aw, bw, real_grid_flat, hamming)
    nc = _get_compiled()

    trace = bool(os.environ.get("BASS_TRACE"))
    res = run_bass_kernel_spmd(nc, in_maps, core_ids=list(range(N_CORES)),
                               trace=trace)
    LAST_EXEC_NS = res.exec_time_ns
    LAST_RESULTS = res.results
    LAST_BKR = res

    full = np.empty((N, N, N), np.float32)               # [z, x, y]
    for c in range(N_CORES):
        full[:, 8 * c:8 * (c + 1), :] = res.results[c]["out"].reshape(N, 8, N)
    o = np.transpose(full, (1, 2, 0))                    # [x, y, z]
    o = (o - o.mean()) / (o.std() + 1e-8)
    return o.astype(np.float32)
